# revision 2
# baseline (speedup 1.0000x reference)
"""MoChA stable chunkwise attention (window w=16) on 8 Trainium2 NeuronCores.

Math: the reference's stabilizing moving-max cancels algebraically:
    P[t] = exp(logits[t])
    S[u] = sum_{v=u-15..u} P[v]          (causal window sum, left-truncated)
    R[u] = emit[u] / S[u]
    out[t] = P[t] * sum_{k=0..15} R[t+k] (anticausal window sum, right-trunc)
Window sums are computed as cumsum (tensor_tensor_scan) + shifted subtract.
logits ~ N(0,1) so exp() never overflows and fp32 cumsum over 1054-element
chunks has no harmful cancellation (verified < 1e-5 rel err vs reference).

Sharding: pure data parallel. B=64 rows -> 8 rows per core. Per core each
row's T=16384 columns split into 16 chunks of 1024 -> 8*16 = 128 SBUF
partitions, 1024 elements each. Host pre-pads inputs with halos so every
chunk can read [a-15, a+1024+15) without edge cases:
    logits padded left+right with -1e30 (exp -> 0)
    emit padded right with 0
"""

import numpy as np

import concourse.bass as bass
import concourse.tile as tile
import concourse.mybir as mybir
from concourse import bacc
from concourse.bass_utils import run_bass_kernel_spmd

F32 = mybir.dt.float32
ALU = mybir.AluOpType
ACTF = mybir.ActivationFunctionType

B, T = 64, 16384
NCORES = 8
RPC = B // NCORES            # rows per core = 8
NCH = 16                     # chunks per row
CH = T // NCH                # 1024 elements per chunk/partition
NPART = RPC * NCH            # 128 partitions
HL = HR = 15                 # halos
LGW = T + HL + HR            # padded logits row length = 16414
EMW = T + HR                 # padded emit row length = 16399

N_STRIPS = 2
W = CH // N_STRIPS           # output elements per strip per partition


def _build_nc():
    nc = bacc.Bacc(
        "TRN2", target_bir_lowering=False, debug=False, num_devices=NCORES
    )
    lg_t = nc.dram_tensor("logits_p", [RPC, LGW], F32, kind="ExternalInput")
    em_t = nc.dram_tensor("emit_p", [RPC, EMW], F32, kind="ExternalInput")
    out_t = nc.dram_tensor("out", [RPC, T], F32, kind="ExternalOutput")

    with tile.TileContext(nc) as tc:
        with (
            tc.tile_pool(name="io", bufs=1) as io_pool,
            tc.tile_pool(name="work", bufs=1) as work_pool,
        ):
            lg_b = io_pool.tile([NPART, CH + 30], F32, tag="lg")
            e_b = io_pool.tile([NPART, CH + 15], F32, tag="em")
            p_b = work_pool.tile([NPART, CH + 30], F32, tag="p")
            c_b = work_pool.tile([NPART, CH + 31], F32, tag="c")
            s_b = work_pool.tile([NPART, CH + 15], F32, tag="s")
            rcp_b = work_pool.tile([NPART, CH + 15], F32, tag="rcp")
            r_b = work_pool.tile([NPART, CH + 15], F32, tag="r")
            d_b = work_pool.tile([NPART, CH + 16], F32, tag="d")
            z_b = work_pool.tile([NPART, CH], F32, tag="z")
            o_b = work_pool.tile([NPART, CH], F32, tag="o")

            # running-state seeds for the two cumsum scans
            nc.vector.memset(c_b[:, 0:1], 0.0)
            nc.vector.memset(d_b[:, 0:1], 0.0)

            for s in range(N_STRIPS):
                ps = 0 if s == 0 else 30 + s * W        # P/logits range [ps, pe)
                pe = 30 + (s + 1) * W
                es = 0 if s == 0 else 15 + s * W        # E/S/R range [es, ee)
                ee = 15 + (s + 1) * W
                os_, oe = s * W, (s + 1) * W            # output range

                # loads (HWDGE)
                nc.sync.dma_start(
                    lg_b[:, ps:pe],
                    bass.AP(lg_t, ps, [[LGW, RPC], [CH, NCH], [1, pe - ps]]),
                )
                nc.sync.dma_start(
                    e_b[:, es:ee],
                    bass.AP(em_t, es, [[EMW, RPC], [CH, NCH], [1, ee - es]]),
                )

                # P = exp(logits)
                nc.scalar.activation(p_b[:, ps:pe], lg_b[:, ps:pe], ACTF.Exp)
                # C = cumsum(P)  (chained across strips via initial)
                nc.vector.tensor_tensor_scan(
                    c_b[:, ps + 1 : pe + 1],
                    p_b[:, ps:pe],
                    p_b[:, ps:pe],
                    c_b[:, ps : ps + 1],
                    ALU.add,
                    ALU.bypass,
                )
                # S[u] = C[u] - C[u-16]
                nc.vector.tensor_sub(
                    s_b[:, es:ee], c_b[:, es + 16 : ee + 16], c_b[:, es:ee]
                )
                # 1/S  (~51 ULP)
                nc.vector.reciprocal_approx_fast(rcp_b[:, es:ee], s_b[:, es:ee])
                # R = emit * (1/S)
                nc.gpsimd.tensor_mul(r_b[:, es:ee], e_b[:, es:ee], rcp_b[:, es:ee])
                # D = cumsum(R)
                nc.vector.tensor_tensor_scan(
                    d_b[:, es + 1 : ee + 1],
                    r_b[:, es:ee],
                    r_b[:, es:ee],
                    d_b[:, es : es + 1],
                    ALU.add,
                    ALU.bypass,
                )
                # Z[t] = D[t+15] - D[t-1]
                nc.gpsimd.tensor_sub(
                    z_b[:, os_:oe], d_b[:, os_ + 16 : oe + 16], d_b[:, os_:oe]
                )
                # out = P * Z
                nc.gpsimd.tensor_mul(
                    o_b[:, os_:oe], p_b[:, os_ + 15 : oe + 15], z_b[:, os_:oe]
                )
                # store
                nc.sync.dma_start(
                    bass.AP(out_t, os_, [[T, RPC], [CH, NCH], [1, W]]),
                    o_b[:, os_:oe],
                )

    nc.compile()
    return nc


_NC_CACHE = None


def _get_nc():
    global _NC_CACHE
    if _NC_CACHE is None:
        _NC_CACHE = _build_nc()
    return _NC_CACHE


def _make_in_maps(emit_probs, softmax_logits):
    emit_probs = np.ascontiguousarray(emit_probs, dtype=np.float32)
    softmax_logits = np.ascontiguousarray(softmax_logits, dtype=np.float32)
    in_maps = []
    for k in range(NCORES):
        rows = slice(k * RPC, (k + 1) * RPC)
        lg_p = np.full((RPC, LGW), -1e30, np.float32)
        lg_p[:, HL : HL + T] = softmax_logits[rows]
        em_p = np.zeros((RPC, EMW), np.float32)
        em_p[:, :T] = emit_probs[rows]
        in_maps.append({"logits_p": lg_p, "emit_p": em_p})
    return in_maps


def run(emit_probs, softmax_logits, trace=False, **kwargs):
    nc = _get_nc()
    in_maps = _make_in_maps(emit_probs, softmax_logits)
    res = run_bass_kernel_spmd(
        nc, in_maps, core_ids=list(range(NCORES)), trace=trace, **kwargs
    )
    out = np.concatenate([res.results[k]["out"] for k in range(NCORES)], axis=0)
    return out, res


def kernel(emit_probs, softmax_logits):
    return run(emit_probs, softmax_logits)[0]


# revision 3
# speedup vs baseline: 1.0660x; 1.0660x over previous
"""MoChA stable chunkwise attention (window w=16) on 8 Trainium2 NeuronCores.

Math: the reference's stabilizing moving-max cancels algebraically:
    P[t] = exp(logits[t])
    S[u] = sum_{v=u-15..u} P[v]          (causal window sum, left-truncated)
    R[u] = emit[u] / S[u]
    out[t] = P[t] * sum_{k=0..15} R[t+k] (anticausal window sum, right-trunc)
Window sums are computed as cumsum (tensor_tensor_scan) + shifted subtract.
logits ~ N(0,1) so exp() never overflows and fp32 cumsum over 1054-element
chunks has no harmful cancellation (verified < 1e-5 rel err vs reference).

Sharding: pure data parallel. B=64 rows -> 8 rows per core. Per core each
row's T=16384 columns split into 16 chunks of 1024 -> 8*16 = 128 SBUF
partitions, 1024 elements each. Host pre-pads inputs with halos so every
chunk can read [a-15, a+1024+15) without edge cases:
    logits padded left+right with -1e30 (exp -> 0)
    emit padded right with 0
"""

import os
import numpy as np

import concourse.bass as bass
import concourse.tile as tile
import concourse.mybir as mybir
from concourse import bacc
from concourse.bass_utils import run_bass_kernel_spmd

F32 = mybir.dt.float32
ALU = mybir.AluOpType
ACTF = mybir.ActivationFunctionType

B, T = 64, 16384
NCORES = 8
RPC = B // NCORES            # rows per core = 8
NCH = 16                     # chunks per row
CH = T // NCH                # 1024 elements per chunk/partition
NPART = RPC * NCH            # 128 partitions
HL = HR = 15                 # halos
LGW = T + HL + HR            # padded logits row length = 16414
EMW = T + HR                 # padded emit row length = 16399

N_STRIPS = int(os.environ.get("K_STRIPS", "4"))
W = CH // N_STRIPS           # output elements per strip per partition
# engine for each op: v=vector, g=gpsimd  (scalar fixed for exp)
ASSIGN = os.environ.get("K_ASSIGN", "vgvvgvg")  # scanC,subS,recip,mulR,scanD,subZ,mulOut
N_LOAD_SPLIT = int(os.environ.get("K_LOADSPLIT", "2"))


def _build_nc():
    nc = bacc.Bacc(
        "TRN2", target_bir_lowering=False, debug=False, num_devices=NCORES
    )
    lg_t = nc.dram_tensor("logits_p", [RPC, LGW], F32, kind="ExternalInput")
    em_t = nc.dram_tensor("emit_p", [RPC, EMW], F32, kind="ExternalInput")
    out_t = nc.dram_tensor("out", [RPC, T], F32, kind="ExternalOutput")

    def eng(i):
        return {"v": nc.vector, "g": nc.gpsimd}[ASSIGN[i]]

    with tile.TileContext(nc) as tc:
        with (
            tc.tile_pool(name="io", bufs=1) as io_pool,
            tc.tile_pool(name="work", bufs=1) as work_pool,
        ):
            lg_b = io_pool.tile([NPART, CH + 30], F32, tag="lg")
            e_b = io_pool.tile([NPART, CH + 15], F32, tag="em")
            p_b = work_pool.tile([NPART, CH + 30], F32, tag="p")
            c_b = work_pool.tile([NPART, CH + 31], F32, tag="c")
            s_b = work_pool.tile([NPART, CH + 15], F32, tag="s")
            rcp_b = work_pool.tile([NPART, CH + 15], F32, tag="rcp")
            r_b = work_pool.tile([NPART, CH + 15], F32, tag="r")
            d_b = work_pool.tile([NPART, CH + 16], F32, tag="d")
            z_b = work_pool.tile([NPART, CH], F32, tag="z")
            o_b = work_pool.tile([NPART, CH], F32, tag="o")

            # running-state seeds for the two cumsum scans
            nc.vector.memset(c_b[:, 0:1], 0.0)
            nc.vector.memset(d_b[:, 0:1], 0.0)

            # loads: N_LOAD_SPLIT DMAs per tensor, aligned to compute strips
            spl = N_STRIPS // N_LOAD_SPLIT  # strips per load chunk
            for h in range(N_LOAD_SPLIT):
                ps = 0 if h == 0 else 30 + h * spl * W
                pe = 30 + (h + 1) * spl * W
                es = 0 if h == 0 else 15 + h * spl * W
                ee = 15 + (h + 1) * spl * W
                nc.sync.dma_start(
                    lg_b[:, ps:pe],
                    bass.AP(lg_t, ps, [[LGW, RPC], [CH, NCH], [1, pe - ps]]),
                )
                nc.sync.dma_start(
                    e_b[:, es:ee],
                    bass.AP(em_t, es, [[EMW, RPC], [CH, NCH], [1, ee - es]]),
                )

            for s in range(N_STRIPS):
                ps = 0 if s == 0 else 30 + s * W        # P/logits range [ps, pe)
                pe = 30 + (s + 1) * W
                es = 0 if s == 0 else 15 + s * W        # E/S/R range [es, ee)
                ee = 15 + (s + 1) * W
                os_, oe = s * W, (s + 1) * W            # output range

                # P = exp(logits)
                nc.scalar.activation(p_b[:, ps:pe], lg_b[:, ps:pe], ACTF.Exp)
                # C = cumsum(P)  (chained across strips via initial)
                eng(0).tensor_tensor_scan(
                    c_b[:, ps + 1 : pe + 1],
                    p_b[:, ps:pe],
                    p_b[:, ps:pe],
                    c_b[:, ps : ps + 1],
                    ALU.add,
                    ALU.bypass,
                )
                # S[u] = C[u] - C[u-16]
                eng(1).tensor_sub(
                    s_b[:, es:ee], c_b[:, es + 16 : ee + 16], c_b[:, es:ee]
                )
                # 1/S  (~51 ULP)  (custom DVE op: vector only)
                nc.vector.reciprocal_approx_fast(rcp_b[:, es:ee], s_b[:, es:ee])
                # R = emit * (1/S)
                eng(3).tensor_mul(r_b[:, es:ee], e_b[:, es:ee], rcp_b[:, es:ee])
                # D = cumsum(R)
                eng(4).tensor_tensor_scan(
                    d_b[:, es + 1 : ee + 1],
                    r_b[:, es:ee],
                    r_b[:, es:ee],
                    d_b[:, es : es + 1],
                    ALU.add,
                    ALU.bypass,
                )
                # Z[t] = D[t+15] - D[t-1]
                eng(5).tensor_sub(
                    z_b[:, os_:oe], d_b[:, os_ + 16 : oe + 16], d_b[:, os_:oe]
                )
                # out = P * Z
                eng(6).tensor_mul(
                    o_b[:, os_:oe], p_b[:, os_ + 15 : oe + 15], z_b[:, os_:oe]
                )
                # store (issued from scalar HWDGE queue to offload sync)
                nc.scalar.dma_start(
                    bass.AP(out_t, os_, [[T, RPC], [CH, NCH], [1, W]]),
                    o_b[:, os_:oe],
                )

    nc.compile()
    return nc


_NC_CACHE = None


def _get_nc():
    global _NC_CACHE
    if _NC_CACHE is None:
        _NC_CACHE = _build_nc()
    return _NC_CACHE


def _make_in_maps(emit_probs, softmax_logits):
    emit_probs = np.ascontiguousarray(emit_probs, dtype=np.float32)
    softmax_logits = np.ascontiguousarray(softmax_logits, dtype=np.float32)
    in_maps = []
    for k in range(NCORES):
        rows = slice(k * RPC, (k + 1) * RPC)
        lg_p = np.full((RPC, LGW), -1e30, np.float32)
        lg_p[:, HL : HL + T] = softmax_logits[rows]
        em_p = np.zeros((RPC, EMW), np.float32)
        em_p[:, :T] = emit_probs[rows]
        in_maps.append({"logits_p": lg_p, "emit_p": em_p})
    return in_maps


def run(emit_probs, softmax_logits, trace=False, **kwargs):
    nc = _get_nc()
    in_maps = _make_in_maps(emit_probs, softmax_logits)
    res = run_bass_kernel_spmd(
        nc, in_maps, core_ids=list(range(NCORES)), trace=trace, **kwargs
    )
    out = np.concatenate([res.results[k]["out"] for k in range(NCORES)], axis=0)
    return out, res


def kernel(emit_probs, softmax_logits):
    return run(emit_probs, softmax_logits)[0]


# revision 5
# speedup vs baseline: 1.0809x; 1.0139x over previous
"""MoChA stable chunkwise attention (window w=16) on 8 Trainium2 NeuronCores.

Math: the reference's stabilizing moving-max cancels algebraically:
    P[t] = exp(logits[t])
    S[u] = sum_{v=u-15..u} P[v]          (causal window sum, left-truncated)
    R[u] = emit[u] / S[u]
    out[t] = P[t] * sum_{k=0..15} R[t+k] (anticausal window sum, right-trunc)
Window sums are computed as cumsum (tensor_tensor_scan) + shifted subtract.
logits ~ N(0,1) so exp() never overflows and fp32 cumsum over 1054-element
chunks has no harmful cancellation (verified < 1e-5 rel err vs reference).

Sharding: pure data parallel. B=64 rows -> 8 rows per core. Per core each
row's T=16384 columns split into 16 chunks of 1024 -> 8*16 = 128 SBUF
partitions, 1024 elements each. Host pre-pads inputs with halos so every
chunk can read [a-15, a+1024+15) without edge cases:
    logits padded left+right with -1e30 (exp -> 0)
    emit padded right with 0
"""

import os
import numpy as np

import concourse.bass as bass
import concourse.tile as tile
import concourse.mybir as mybir
from concourse import bacc
from concourse.bass_utils import run_bass_kernel_spmd

F32 = mybir.dt.float32
ALU = mybir.AluOpType
ACTF = mybir.ActivationFunctionType

B, T = 64, 16384
NCORES = 8
RPC = B // NCORES            # rows per core = 8
NCH = 16                     # chunks per row
CH = T // NCH                # 1024 elements per chunk/partition
NPART = RPC * NCH            # 128 partitions
HL = HR = 15                 # halos
LGW = T + HL + HR            # padded logits row length = 16414
EMW = T + HR                 # padded emit row length = 16399

N_STRIPS = int(os.environ.get("K_STRIPS", "4"))
W = CH // N_STRIPS           # output elements per strip per partition
# engine for each op: v=vector, g=gpsimd  (scalar fixed for exp)
ASSIGN = os.environ.get("K_ASSIGN", "vgvvgvg")  # scanC,subS,recip,mulR,scanD,subZ,mulOut
N_LOAD_SPLIT = int(os.environ.get("K_LOADSPLIT", "2"))


def _build_nc():
    nc = bacc.Bacc(
        "TRN2", target_bir_lowering=False, debug=False, num_devices=NCORES
    )
    lg_t = nc.dram_tensor("logits_p", [RPC, LGW], F32, kind="ExternalInput")
    em_t = nc.dram_tensor("emit_p", [RPC, EMW], F32, kind="ExternalInput")
    out_t = nc.dram_tensor("out", [RPC, T], F32, kind="ExternalOutput")

    def eng(i):
        return {"v": nc.vector, "g": nc.gpsimd}[ASSIGN[i]]

    with tile.TileContext(nc) as tc:
        with (
            tc.tile_pool(name="io", bufs=1) as io_pool,
            tc.tile_pool(name="work", bufs=1) as work_pool,
        ):
            lg_b = io_pool.tile([NPART, CH + 30], F32, tag="lg")
            e_b = io_pool.tile([NPART, CH + 15], F32, tag="em")
            p_b = work_pool.tile([NPART, CH + 30], F32, tag="p")
            c_b = work_pool.tile([NPART, CH + 31], F32, tag="c")
            s_b = work_pool.tile([NPART, CH + 15], F32, tag="s")
            rcp_b = work_pool.tile([NPART, CH + 15], F32, tag="rcp")
            r_b = work_pool.tile([NPART, CH + 15], F32, tag="r")
            d_b = work_pool.tile([NPART, CH + 16], F32, tag="d")
            z_b = work_pool.tile([NPART, CH], F32, tag="z")
            o_b = work_pool.tile([NPART, CH], F32, tag="o")

            # running-state seeds for the two cumsum scans
            nc.vector.memset(c_b[:, 0:1], 0.0)
            nc.vector.memset(d_b[:, 0:1], 0.0)

            # loads: N_LOAD_SPLIT DMAs per tensor, aligned to compute strips
            spl = N_STRIPS // N_LOAD_SPLIT  # strips per load chunk
            for h in range(N_LOAD_SPLIT):
                ps = 0 if h == 0 else 30 + h * spl * W
                pe = 30 + (h + 1) * spl * W
                es = 0 if h == 0 else 15 + h * spl * W
                ee = 15 + (h + 1) * spl * W
                nc.sync.dma_start(
                    lg_b[:, ps:pe],
                    bass.AP(lg_t, ps, [[LGW, RPC], [CH, NCH], [1, pe - ps]]),
                )
                nc.sync.dma_start(
                    e_b[:, es:ee],
                    bass.AP(em_t, es, [[EMW, RPC], [CH, NCH], [1, ee - es]]),
                )

            for s in range(N_STRIPS):
                ps = 0 if s == 0 else 30 + s * W        # P/logits range [ps, pe)
                pe = 30 + (s + 1) * W
                es = 0 if s == 0 else 15 + s * W        # E/S/R range [es, ee)
                ee = 15 + (s + 1) * W
                os_, oe = s * W, (s + 1) * W            # output range

                # P = exp(logits)
                nc.scalar.activation(p_b[:, ps:pe], lg_b[:, ps:pe], ACTF.Exp)
                # C = cumsum(P)  (chained across strips via initial)
                eng(0).tensor_tensor_scan(
                    c_b[:, ps + 1 : pe + 1],
                    p_b[:, ps:pe],
                    p_b[:, ps:pe],
                    c_b[:, ps : ps + 1],
                    ALU.add,
                    ALU.bypass,
                )
                # S[u] = C[u] - C[u-16]
                eng(1).tensor_sub(
                    s_b[:, es:ee], c_b[:, es + 16 : ee + 16], c_b[:, es:ee]
                )
                # 1/S  (~51 ULP)  (custom DVE op: vector only)
                nc.vector.reciprocal_approx_fast(rcp_b[:, es:ee], s_b[:, es:ee])
                # R = emit * (1/S)
                eng(3).tensor_mul(r_b[:, es:ee], e_b[:, es:ee], rcp_b[:, es:ee])
                # D = cumsum(R)
                eng(4).tensor_tensor_scan(
                    d_b[:, es + 1 : ee + 1],
                    r_b[:, es:ee],
                    r_b[:, es:ee],
                    d_b[:, es : es + 1],
                    ALU.add,
                    ALU.bypass,
                )
                # Z[t] = D[t+15] - D[t-1]
                eng(5).tensor_sub(
                    z_b[:, os_:oe], d_b[:, os_ + 16 : oe + 16], d_b[:, os_:oe]
                )
                # out = P * Z
                eng(6).tensor_mul(
                    o_b[:, os_:oe], p_b[:, os_ + 15 : oe + 15], z_b[:, os_:oe]
                )
                # store (issued from scalar HWDGE queue to offload sync)
                nc.scalar.dma_start(
                    bass.AP(out_t, os_, [[T, RPC], [CH, NCH], [1, W]]),
                    o_b[:, os_:oe],
                )

    nc.compile()
    return nc


_NC_CACHE = None
IMPL = os.environ.get("K_IMPL", "scan")


def _get_nc():
    global _NC_CACHE
    if _NC_CACHE is None:
        if IMPL == "mm":
            import kernel_mm

            _NC_CACHE = kernel_mm.build_nc()
        else:
            _NC_CACHE = _build_nc()
    return _NC_CACHE


def _make_in_maps(emit_probs, softmax_logits):
    emit_probs = np.ascontiguousarray(emit_probs, dtype=np.float32)
    softmax_logits = np.ascontiguousarray(softmax_logits, dtype=np.float32)
    in_maps = []
    for k in range(NCORES):
        rows = slice(k * RPC, (k + 1) * RPC)
        lg_p = np.full((RPC, LGW), -1e30, np.float32)
        lg_p[:, HL : HL + T] = softmax_logits[rows]
        em_p = np.zeros((RPC, EMW), np.float32)
        em_p[:, :T] = emit_probs[rows]
        in_maps.append({"logits_p": lg_p, "emit_p": em_p})
    return in_maps


def run(emit_probs, softmax_logits, trace=False, **kwargs):
    nc = _get_nc()
    if IMPL == "mm":
        import kernel_mm

        in_maps = kernel_mm.make_in_maps(emit_probs, softmax_logits)
    else:
        in_maps = _make_in_maps(emit_probs, softmax_logits)
    res = run_bass_kernel_spmd(
        nc, in_maps, core_ids=list(range(NCORES)), trace=trace, **kwargs
    )
    out = np.concatenate([res.results[k]["out"] for k in range(NCORES)], axis=0)
    return out, res


def kernel(emit_probs, softmax_logits):
    return run(emit_probs, softmax_logits)[0]
